# revision 24
# baseline (speedup 1.0000x reference)
"""Trainium2 Bass kernel for 2-head causal self-attention.

Problem: embedded [B=4, S=2048, E=1024], Wq/Wk/Wv [H=2, E, HD=512].
out[b, s, h*HD:(h+1)*HD] = softmax(causal(Q K^T / sqrt(HD))) @ V for head h.

Sharding: 8 (b, h) pairs -> 8 cores, one pair each (perfect SPMD balance).

Per-core dataflow (bf16 operands everywhere; psum accumulates f32):
  - Host passes X^T (so E is on the partition/contraction axis directly).
  - Phase 1: QT[d,q], KT[d,q] (W stationary, X^T moving) and V[k,d]
    (X^T stationary, W moving). Within each group of 4 psum chains the
    contraction (eo) is the OUTER loop so at startup the PE consumes DMA
    chunks as they land instead of stalling on one chain's last chunk.
    DMA descriptors are issued from sync+gpsimd+scalar in parallel
    (descriptor issue ~0.6us each is the startup bottleneck); warmup
    matmuls on a scratch tile keep the PE ramping until data lands.
  - Phase 2: scores computed TRANSPOSED: scoresT[k, q-chunk] =
    (KT tile).T @ QT with exact causal widths per diagonal tile. After
    exp, the attnT tile [k, q] is the stationary operand for
    ctx[q, d] += attnT.T @ V[k, d] -- no transpose ever materialized.
  - Softmax denominators come FREE from the ctx matmul: V carries a
    leading all-ones column, so ctx psum column 0 accumulates
    sum_k attnT[k, q] = the denominator, per q-partition. The 513-wide
    ctx output is split 257+256 across two psum banks (a bank holds 512
    f32). No row-sum or rotation matmuls on the PE at all.
  - Causal masking: only the 128x128 diagonal block of each diag tile is
    triangular; a single [128,128] 0/1 bf16 mask multiply handles it.
  - ctx chains run s ASCENDING interleaved with the last few score
    tiles, so the PE never waits on exp of the final tile and the kernel
    tail is one short chain + two 64KB DMAs.
  - ONE psum pool (8 banks, one tag) spans both phases: a pool close
    between phases would barrier the PE on all outstanding psum->SBUF
    copies (~2us). V is split per q-chunk so M=0's ctx doesn't depend on
    the last V copies either.
"""

import ml_dtypes
import numpy as np

import concourse.bass as bass
import concourse.mybir as mybir
from concourse import bacc
import concourse.tile as tile
from concourse import bass_utils

B, S, E, H, HD = 4, 2048, 1024, 2, 512
P = 128
EO = E // P          # 8 e-tiles (contraction for QKV)
DT = HD // P         # 4 d-tiles (contraction for scores)
NKT = S // P         # 16 k-tiles
NSUP = S // 512      # 4 q super-tiles (512 wide)
SCALE = float(HD) ** -0.5
F32 = mybir.dt.float32
EXP = mybir.ActivationFunctionType.Exp
BF16 = mybir.dt.bfloat16

_NC = None


def _body(tc, xt_d, wq_d, wk_d, wv_d, mask_d, out_d):
    nc = tc.nc

    import contextlib

    with contextlib.ExitStack() as ctx:
        per = ctx.enter_context(tc.tile_pool(name="per", bufs=1))
        # Persistent SBUF: QT/KT as [d_inner=128, d_tile, q], V per q-chunk
        # as [k_inner, k_tile, 1+d] (col 0 = ones for the denominator).
        qt = per.tile([P, DT, S], BF16)
        kt = per.tile([P, DT, S], BF16)
        vq = [per.tile([P, 4, HD + 1], BF16, name=f"v{i}") for i in range(4)]
        mask_sb = per.tile([P, P], BF16)
        wsrc = per.tile([P, P], BF16)
        psall = ctx.enter_context(tc.tile_pool(name="psall", bufs=8, space="PSUM"))

        def ps_tile(name):
            return psall.tile([P, 512], F32, tag="ps", name=name)

        # ---------------- Phase 1: QT, KT, V projections ----------------
        with (
            tc.tile_pool(name="wpool", bufs=1) as wpool,
            tc.tile_pool(name="xpool", bufs=2) as xpool,
        ):
            # PE warmup: ramp the p-state during the initial DMA wait with
            # matmuls on a zeroed scratch tile (never read back).
            nc.gpsimd.memset(wsrc, 0.0)
            wps = ps_tile("wps")
            for _ in range(38):
                nc.tensor.matmul(wps[:, 0:P], lhsT=wsrc, rhs=wsrc,
                                 start=True, stop=True)

            wq_sb = wpool.tile([P, EO, HD], BF16)
            wk_sb = wpool.tile([P, EO, HD], BF16)
            wv_sb = wpool.tile([P, EO, HD], BF16)
            wq_r = wq_d.rearrange("(eo p) d -> p eo d", p=P)
            wk_r = wk_d.rearrange("(eo p) d -> p eo d", p=P)
            wv_r = wv_d.rearrange("(eo p) d -> p eo d", p=P)

            # DMA descriptor issue is ~0.6us/desc on an engine queue; spread
            # the startup-critical ones across three engines so transfers
            # overlap the PE chase instead of serializing behind one queue.
            # Issue order must match PE consumption order (round eo needs
            # wq[eo]+xc0[eo]); anything queued early steals wire bandwidth
            # from the chase, so wk/wv/mask go AFTER the xc0 stream on sync
            # (its descriptor pacing naturally lands them just before the
            # KT / V groups need them).
            xc0 = xpool.tile([P, EO, 512], BF16, tag="xc", name="xc")
            for eo in range(EO):
                eng = nc.scalar if eo % 2 == 0 else nc.gpsimd
                eng.dma_start(out=wq_sb[:, eo, :], in_=wq_r[:, eo, :])
                nc.sync.dma_start(out=xc0[:, eo, :], in_=xt_d[eo, 0, :, :])
            for eo in range(EO):
                nc.sync.dma_start(out=wk_sb[:, eo, :], in_=wk_r[:, eo, :])
            for eo in range(EO):
                nc.sync.dma_start(out=wv_sb[:, eo, :], in_=wv_r[:, eo, :])
            nc.sync.dma_start(out=mask_sb, in_=mask_d)
            # ones column of V (column 0 of every k-tile)
            for i in range(4):
                nc.gpsimd.memset(vq[i][:, :, 0:1], 1.0)

            for qc in range(4):  # 512-wide q/k chunk
                if qc == 0:
                    xc = xc0
                else:
                    xc = xpool.tile([P, EO, 512], BF16, tag="xc", name="xc")
                    for eo in range(EO):
                        nc.sync.dma_start(
                            out=xc[:, eo, :], in_=xt_d[eo, qc, :, :]
                        )

                # QT / KT: out[d_tile, q-chunk] = sum_e W[e, d].T @ XT[e, q]
                # 4 chains per group, eo outer (DMA-chase friendly).
                for w_sb, dst, eng in ((wq_sb, qt, "s"), (wk_sb, kt, "v")):
                    ps_g = [ps_tile(f"ps{dm}") for dm in range(DT)]
                    for eo in range(EO):
                        for dm in range(DT):
                            nc.tensor.matmul(
                                ps_g[dm],
                                lhsT=w_sb[:, eo, dm * P : (dm + 1) * P],
                                rhs=xc[:, eo, :],
                                start=(eo == 0),
                                stop=(eo == EO - 1),
                            )
                    for dm in range(DT):
                        dslice = dst[:, dm, qc * 512 : (qc + 1) * 512]
                        if eng == "s":
                            nc.scalar.copy(dslice, ps_g[dm])
                        else:
                            nc.vector.tensor_copy(dslice, ps_g[dm])

                # V: out[k_tile, d] = sum_e XT[e, k].T @ Wv[e, d]
                ps_v = [ps_tile(f"psv{ki}") for ki in range(4)]
                for eo in range(EO):
                    for ki in range(4):
                        nc.tensor.matmul(
                            ps_v[ki],
                            lhsT=xc[:, eo, ki * P : (ki + 1) * P],
                            rhs=wv_sb[:, eo, :],
                            start=(eo == 0),
                            stop=(eo == EO - 1),
                        )
                for ki in range(4):
                    # (gpsimd cannot read PSUM) pair the copies per engine so
                    # both Act and DVE free up ~2 copies after the last V
                    # round -- phase 2's first exp/mask need them quickly.
                    dv = vq[qc][:, ki, 1 : HD + 1]
                    if (ki < 2) if qc == 3 else (ki % 2 == 0):
                        nc.vector.tensor_copy(dv, ps_v[ki])
                    else:
                        nc.scalar.copy(dv, ps_v[ki])

        # ---------------- Phase 2: attention ----------------
        with (
            tc.tile_pool(name="apool", bufs=2) as apool,
            tc.tile_pool(name="opool", bufs=3) as opool,
        ):
            for M in range(NSUP):  # q super-tile: q in [512M, 512(M+1))
                at = apool.tile([P, NKT, 512], BF16, tag="at")

                def score_tile(j, at=at, M=M):
                    r = j - 4 * M
                    off = P * r if r > 0 else 0
                    ps = ps_tile("ps_s")
                    for dt_i in range(DT):
                        nc.tensor.matmul(
                            ps[:, off:512],
                            lhsT=kt[:, dt_i, j * P : (j + 1) * P],
                            rhs=qt[:, dt_i, M * 512 + off : (M + 1) * 512],
                            start=(dt_i == 0),
                            stop=(dt_i == DT - 1),
                        )
                    a_j = at[:, j, off:512]
                    # attnT[k, q] = exp(scoresT / sqrt(hd)); then zero the
                    # triangular invalid part of the 128-wide diagonal block.
                    nc.scalar.activation(a_j, ps[:, off:512], EXP, scale=SCALE)
                    if r >= 0:
                        blk = at[:, j, off : off + P]
                        nc.vector.tensor_mul(blk, blk, mask_sb)

                def ctx_pair(s, at=at, M=M):
                    # ctx[q_sub, d] += attnT_tile.T @ V, split 257+256 so the
                    # ones column fits: psA col 0 = softmax denominator.
                    nj = 4 * M + s + 1
                    psA = ps_tile("psA")
                    psB = ps_tile("psB")
                    for j in range(nj):
                        nc.tensor.matmul(
                            psA[:, 0:257],
                            lhsT=at[:, j, s * P : (s + 1) * P],
                            rhs=vq[j // 4][:, j % 4, 0:257],
                            start=(j == 0),
                            stop=(j == nj - 1),
                        )
                    for j in range(nj):
                        nc.tensor.matmul(
                            psB[:, 0:256],
                            lhsT=at[:, j, s * P : (s + 1) * P],
                            rhs=vq[j // 4][:, j % 4, 257:513],
                            start=(j == 0),
                            stop=(j == nj - 1),
                        )
                    rinv = opool.tile([P, 1], F32, tag="rinv", name="rinv")
                    nc.vector.reciprocal(rinv, psA[:, 0:1])
                    o_sb = opool.tile([P, HD], BF16, tag="o", name="o_sb")
                    row0 = M * 512 + s * P
                    # output muls live on DVE so the Act queue holds only
                    # exps (a mul between exps delays the exp the next ctx
                    # chain waits on); final pair keeps Act/DVE parallel for
                    # the shortest tail.
                    if M == 3 and s == 3:
                        nc.scalar.mul(o_sb[:, 0:256], psA[:, 1:257], rinv)
                    else:
                        nc.vector.tensor_scalar_mul(o_sb[:, 0:256], psA[:, 1:257], rinv)
                    nc.sync.dma_start(
                        out=out_d[row0 : row0 + P, 0:256], in_=o_sb[:, 0:256]
                    )
                    nc.vector.tensor_scalar_mul(o_sb[:, 256:512], psB[:, 0:256], rinv)
                    nc.sync.dma_start(
                        out=out_d[row0 : row0 + P, 256:512], in_=o_sb[:, 256:512]
                    )

                # Interleave: ctx chain s only needs tiles j <= 4M+s, and
                # each exp/mask needs PE work after it to hide its latency,
                # so keep the PE ~1.5us ahead of the tile each ctx needs.
                if M == 0:
                    for j in range(4):
                        score_tile(j)
                    for s in range(4):
                        ctx_pair(s)
                else:
                    for j in range(4 * M + 2):
                        score_tile(j)
                    ctx_pair(0)
                    score_tile(4 * M + 2)
                    ctx_pair(1)
                    score_tile(4 * M + 3)
                    ctx_pair(2)
                    ctx_pair(3)


def _build_nc():
    nc = bacc.Bacc("TRN2", target_bir_lowering=False, debug=False, num_devices=8)
    # xt pre-chunked on host: [eo, qc, p, col] so every xc DMA is a fully
    # contiguous 128KB read (the strided [E,S] layout was the only
    # non-contiguous DMA, and the startup chase is wire-bandwidth-pinned).
    xt_d = nc.dram_tensor("xt", [EO, 4, P, 512], BF16, kind="ExternalInput")
    wq_d = nc.dram_tensor("wq", [E, HD], BF16, kind="ExternalInput")
    wk_d = nc.dram_tensor("wk", [E, HD], BF16, kind="ExternalInput")
    wv_d = nc.dram_tensor("wv", [E, HD], BF16, kind="ExternalInput")
    mask_d = nc.dram_tensor("mask", [P, P], BF16, kind="ExternalInput")
    out_d = nc.dram_tensor("out", [S, HD], BF16, kind="ExternalOutput")
    with tile.TileContext(nc) as tc:
        _body(tc, xt_d.ap(), wq_d.ap(), wk_d.ap(), wv_d.ap(), mask_d.ap(), out_d.ap())
    nc.compile()
    return nc


def _mask_np():
    # mask[k_local, q_local] = 1 iff q_local >= k_local (one diagonal block)
    q = np.arange(P)[None, :]
    k = np.arange(P)[:, None]
    return (q >= k).astype(ml_dtypes.bfloat16)


def _in_maps(embedded, Wq, Wk, Wv):
    embedded = np.asarray(embedded, dtype=np.float32)
    Wq = np.asarray(Wq, dtype=np.float32)
    Wk = np.asarray(Wk, dtype=np.float32)
    Wv = np.asarray(Wv, dtype=np.float32)
    mask = _mask_np()
    in_maps = []
    for core in range(8):
        b, h = divmod(core, 2)
        in_maps.append(
            {
                "xt": np.ascontiguousarray(
                    embedded[b].T.reshape(EO, P, 4, 512).transpose(0, 2, 1, 3)
                ).astype(ml_dtypes.bfloat16),
                "wq": np.ascontiguousarray(Wq[h]).astype(ml_dtypes.bfloat16),
                "wk": np.ascontiguousarray(Wk[h]).astype(ml_dtypes.bfloat16),
                "wv": np.ascontiguousarray(Wv[h]).astype(ml_dtypes.bfloat16),
                "mask": mask,
            }
        )
    return in_maps


def _gather(results):
    out = np.empty((B, S, H * HD), np.float32)
    for core in range(8):
        b, h = divmod(core, 2)
        out[b, :, h * HD : (h + 1) * HD] = results[core]["out"].astype(np.float32)
    return out


def _get_nc():
    global _NC
    if _NC is None:
        _NC = _build_nc()
    return _NC


def kernel(embedded, Wq, Wk, Wv):
    res = bass_utils.run_bass_kernel_spmd(
        _get_nc(), _in_maps(embedded, Wq, Wk, Wv), core_ids=list(range(8))
    )
    return _gather(res.results)


def kernel_traced(embedded, Wq, Wk, Wv):
    """Like kernel() but with NTFF tracing; returns (out, BassKernelResults)."""
    res = bass_utils.run_bass_kernel_spmd(
        _get_nc(), _in_maps(embedded, Wq, Wk, Wv), core_ids=list(range(8)), trace=True
    )
    return _gather(res.results), res


# revision 25
# speedup vs baseline: 1.0024x; 1.0024x over previous
"""Trainium2 Bass kernel for 2-head causal self-attention.

Problem: embedded [B=4, S=2048, E=1024], Wq/Wk/Wv [H=2, E, HD=512].
out[b, s, h*HD:(h+1)*HD] = softmax(causal(Q K^T / sqrt(HD))) @ V for head h.

Sharding: 8 (b, h) pairs -> 8 cores, one pair each (perfect SPMD balance).

Per-core dataflow (bf16 operands everywhere; psum accumulates f32):
  - Host passes X^T (so E is on the partition/contraction axis directly).
  - Phase 1: QT[d,q], KT[d,q] (W stationary, X^T moving) and V[k,d]
    (X^T stationary, W moving). Within each group of 4 psum chains the
    contraction (eo) is the OUTER loop so at startup the PE consumes DMA
    chunks as they land instead of stalling on one chain's last chunk.
    DMA descriptors are issued from sync+gpsimd+scalar in parallel
    (descriptor issue ~0.6us each is the startup bottleneck); warmup
    matmuls on a scratch tile keep the PE ramping until data lands.
  - Phase 2: scores computed TRANSPOSED: scoresT[k, q-chunk] =
    (KT tile).T @ QT with exact causal widths per diagonal tile. After
    exp, the attnT tile [k, q] is the stationary operand for
    ctx[q, d] += attnT.T @ V[k, d] -- no transpose ever materialized.
  - Softmax denominators come FREE from the ctx matmul: V carries a
    leading all-ones column, so ctx psum column 0 accumulates
    sum_k attnT[k, q] = the denominator, per q-partition. The 513-wide
    ctx output is split 257+256 across two psum banks (a bank holds 512
    f32). No row-sum or rotation matmuls on the PE at all.
  - Causal masking: only the 128x128 diagonal block of each diag tile is
    triangular; a single [128,128] 0/1 bf16 mask multiply handles it.
  - ctx chains run s ASCENDING interleaved with the last few score
    tiles, so the PE never waits on exp of the final tile and the kernel
    tail is one short chain + two 64KB DMAs.
  - ONE psum pool (8 banks, one tag) spans both phases: a pool close
    between phases would barrier the PE on all outstanding psum->SBUF
    copies (~2us). V is split per q-chunk so M=0's ctx doesn't depend on
    the last V copies either.
"""

import ml_dtypes
import numpy as np

import concourse.bass as bass
import concourse.mybir as mybir
from concourse import bacc
import concourse.tile as tile
from concourse import bass_utils

B, S, E, H, HD = 4, 2048, 1024, 2, 512
P = 128
EO = E // P          # 8 e-tiles (contraction for QKV)
DT = HD // P         # 4 d-tiles (contraction for scores)
NKT = S // P         # 16 k-tiles
NSUP = S // 512      # 4 q super-tiles (512 wide)
SCALE = float(HD) ** -0.5
F32 = mybir.dt.float32
EXP = mybir.ActivationFunctionType.Exp
BF16 = mybir.dt.bfloat16

_NC = None


def _body(tc, xt_d, wq_d, wk_d, wv_d, mask_d, out_d):
    nc = tc.nc

    import contextlib

    with contextlib.ExitStack() as ctx:
        per = ctx.enter_context(tc.tile_pool(name="per", bufs=1))
        # Persistent SBUF: QT/KT as [d_inner=128, d_tile, q], V per q-chunk
        # as [k_inner, k_tile, 1+d] (col 0 = ones for the denominator).
        qt = per.tile([P, DT, S], BF16)
        kt = per.tile([P, DT, S], BF16)
        vq = [per.tile([P, 4, HD + 1], BF16, name=f"v{i}") for i in range(4)]
        mask_sb = per.tile([P, P], BF16)
        wsrc = per.tile([P, P], BF16)
        psall = ctx.enter_context(tc.tile_pool(name="psall", bufs=8, space="PSUM"))

        def ps_tile(name):
            return psall.tile([P, 512], F32, tag="ps", name=name)

        # ---------------- Phase 1: QT, KT, V projections ----------------
        with (
            tc.tile_pool(name="wpool", bufs=1) as wpool,
            tc.tile_pool(name="xpool", bufs=2) as xpool,
        ):
            # PE warmup: ramp the p-state during the initial DMA wait with
            # matmuls on a zeroed scratch tile (never read back).
            nc.gpsimd.memset(wsrc, 0.0)
            wps = ps_tile("wps")
            for _ in range(38):
                nc.tensor.matmul(wps[:, 0:P], lhsT=wsrc, rhs=wsrc,
                                 start=True, stop=True)

            wq_sb = wpool.tile([P, EO, HD], BF16)
            wk_sb = wpool.tile([P, EO, HD], BF16)
            wv_sb = wpool.tile([P, EO, HD], BF16)
            xt_r = xt_d.rearrange("(eo p) q -> p eo q", p=P)
            wq_r = wq_d.rearrange("(eo p) d -> p eo d", p=P)
            wk_r = wk_d.rearrange("(eo p) d -> p eo d", p=P)
            wv_r = wv_d.rearrange("(eo p) d -> p eo d", p=P)

            # DMA descriptor issue is ~0.6us/desc on an engine queue; spread
            # the startup-critical ones across three engines so transfers
            # overlap the PE chase instead of serializing behind one queue.
            # Issue order must match PE consumption order (round eo needs
            # wq[eo]+xc0[eo]); anything queued early steals wire bandwidth
            # from the chase, so wk/wv/mask go AFTER the xc0 stream on sync
            # (its descriptor pacing naturally lands them just before the
            # KT / V groups need them).
            xc0 = xpool.tile([P, EO, 512], BF16, tag="xc", name="xc")
            for eo in range(EO):
                eng = nc.scalar if eo % 2 == 0 else nc.gpsimd
                eng.dma_start(out=wq_sb[:, eo, :], in_=wq_r[:, eo, :])
                nc.sync.dma_start(out=xc0[:, eo, :], in_=xt_r[:, eo, 0:512])
            for eo in range(EO):
                nc.sync.dma_start(out=wk_sb[:, eo, :], in_=wk_r[:, eo, :])
            for eo in range(EO):
                nc.sync.dma_start(out=wv_sb[:, eo, :], in_=wv_r[:, eo, :])
            nc.sync.dma_start(out=mask_sb, in_=mask_d)
            # ones column of V (column 0 of every k-tile)
            for i in range(4):
                nc.gpsimd.memset(vq[i][:, :, 0:1], 1.0)

            for qc in range(4):  # 512-wide q/k chunk
                if qc == 0:
                    xc = xc0
                else:
                    xc = xpool.tile([P, EO, 512], BF16, tag="xc", name="xc")
                    for eo in range(EO):
                        nc.sync.dma_start(
                            out=xc[:, eo, :],
                            in_=xt_r[:, eo, qc * 512 : (qc + 1) * 512],
                        )

                # QT / KT: out[d_tile, q-chunk] = sum_e W[e, d].T @ XT[e, q]
                # 4 chains per group, eo outer (DMA-chase friendly).
                for w_sb, dst, eng in ((wq_sb, qt, "s"), (wk_sb, kt, "v")):
                    ps_g = [ps_tile(f"ps{dm}") for dm in range(DT)]
                    for eo in range(EO):
                        for dm in range(DT):
                            nc.tensor.matmul(
                                ps_g[dm],
                                lhsT=w_sb[:, eo, dm * P : (dm + 1) * P],
                                rhs=xc[:, eo, :],
                                start=(eo == 0),
                                stop=(eo == EO - 1),
                            )
                    for dm in range(DT):
                        dslice = dst[:, dm, qc * 512 : (qc + 1) * 512]
                        if eng == "s":
                            nc.scalar.copy(dslice, ps_g[dm])
                        else:
                            nc.vector.tensor_copy(dslice, ps_g[dm])

                # V: out[k_tile, d] = sum_e XT[e, k].T @ Wv[e, d]
                ps_v = [ps_tile(f"psv{ki}") for ki in range(4)]
                for eo in range(EO):
                    for ki in range(4):
                        nc.tensor.matmul(
                            ps_v[ki],
                            lhsT=xc[:, eo, ki * P : (ki + 1) * P],
                            rhs=wv_sb[:, eo, :],
                            start=(eo == 0),
                            stop=(eo == EO - 1),
                        )
                for ki in range(4):
                    # (gpsimd cannot read PSUM) pair the copies per engine so
                    # both Act and DVE free up ~2 copies after the last V
                    # round -- phase 2's first exp/mask need them quickly.
                    dv = vq[qc][:, ki, 1 : HD + 1]
                    if (ki < 2) if qc == 3 else (ki % 2 == 0):
                        nc.vector.tensor_copy(dv, ps_v[ki])
                    else:
                        nc.scalar.copy(dv, ps_v[ki])

        # ---------------- Phase 2: attention ----------------
        with (
            tc.tile_pool(name="apool", bufs=2) as apool,
            tc.tile_pool(name="opool", bufs=3) as opool,
        ):
            for M in range(NSUP):  # q super-tile: q in [512M, 512(M+1))
                at = apool.tile([P, NKT, 512], BF16, tag="at")

                def score_tile(j, at=at, M=M):
                    r = j - 4 * M
                    off = P * r if r > 0 else 0
                    ps = ps_tile("ps_s")
                    for dt_i in range(DT):
                        nc.tensor.matmul(
                            ps[:, off:512],
                            lhsT=kt[:, dt_i, j * P : (j + 1) * P],
                            rhs=qt[:, dt_i, M * 512 + off : (M + 1) * 512],
                            start=(dt_i == 0),
                            stop=(dt_i == DT - 1),
                        )
                    a_j = at[:, j, off:512]
                    # attnT[k, q] = exp(scoresT / sqrt(hd)); then zero the
                    # triangular invalid part of the 128-wide diagonal block.
                    nc.scalar.activation(a_j, ps[:, off:512], EXP, scale=SCALE)
                    if r >= 0:
                        blk = at[:, j, off : off + P]
                        nc.vector.tensor_mul(blk, blk, mask_sb)

                def ctx_pair(s, at=at, M=M):
                    # ctx[q_sub, d] += attnT_tile.T @ V, split 257+256 so the
                    # ones column fits: psA col 0 = softmax denominator.
                    nj = 4 * M + s + 1
                    psA = ps_tile("psA")
                    psB = ps_tile("psB")
                    for j in range(nj):
                        nc.tensor.matmul(
                            psA[:, 0:257],
                            lhsT=at[:, j, s * P : (s + 1) * P],
                            rhs=vq[j // 4][:, j % 4, 0:257],
                            start=(j == 0),
                            stop=(j == nj - 1),
                        )
                    for j in range(nj):
                        nc.tensor.matmul(
                            psB[:, 0:256],
                            lhsT=at[:, j, s * P : (s + 1) * P],
                            rhs=vq[j // 4][:, j % 4, 257:513],
                            start=(j == 0),
                            stop=(j == nj - 1),
                        )
                    rinv = opool.tile([P, 1], F32, tag="rinv", name="rinv")
                    nc.vector.reciprocal(rinv, psA[:, 0:1])
                    o_sb = opool.tile([P, HD], BF16, tag="o", name="o_sb")
                    row0 = M * 512 + s * P
                    # output muls live on DVE so the Act queue holds only
                    # exps (a mul between exps delays the exp the next ctx
                    # chain waits on); final pair keeps Act/DVE parallel for
                    # the shortest tail.
                    if M == 3 and s == 3:
                        nc.scalar.mul(o_sb[:, 0:256], psA[:, 1:257], rinv)
                    else:
                        nc.vector.tensor_scalar_mul(o_sb[:, 0:256], psA[:, 1:257], rinv)
                    nc.sync.dma_start(
                        out=out_d[row0 : row0 + P, 0:256], in_=o_sb[:, 0:256]
                    )
                    nc.vector.tensor_scalar_mul(o_sb[:, 256:512], psB[:, 0:256], rinv)
                    nc.sync.dma_start(
                        out=out_d[row0 : row0 + P, 256:512], in_=o_sb[:, 256:512]
                    )

                # Interleave: ctx chain s only needs tiles j <= 4M+s, and
                # each exp/mask needs PE work after it to hide its latency,
                # so keep the PE ~1.5us ahead of the tile each ctx needs.
                if M == 0:
                    for j in range(4):
                        score_tile(j)
                    for s in range(4):
                        ctx_pair(s)
                else:
                    for j in range(4 * M + 2):
                        score_tile(j)
                    ctx_pair(0)
                    score_tile(4 * M + 2)
                    ctx_pair(1)
                    score_tile(4 * M + 3)
                    ctx_pair(2)
                    ctx_pair(3)


def _build_nc():
    nc = bacc.Bacc("TRN2", target_bir_lowering=False, debug=False, num_devices=8)
    xt_d = nc.dram_tensor("xt", [E, S], BF16, kind="ExternalInput")
    wq_d = nc.dram_tensor("wq", [E, HD], BF16, kind="ExternalInput")
    wk_d = nc.dram_tensor("wk", [E, HD], BF16, kind="ExternalInput")
    wv_d = nc.dram_tensor("wv", [E, HD], BF16, kind="ExternalInput")
    mask_d = nc.dram_tensor("mask", [P, P], BF16, kind="ExternalInput")
    out_d = nc.dram_tensor("out", [S, HD], BF16, kind="ExternalOutput")
    with tile.TileContext(nc) as tc:
        _body(tc, xt_d.ap(), wq_d.ap(), wk_d.ap(), wv_d.ap(), mask_d.ap(), out_d.ap())
    nc.compile()
    return nc


def _mask_np():
    # mask[k_local, q_local] = 1 iff q_local >= k_local (one diagonal block)
    q = np.arange(P)[None, :]
    k = np.arange(P)[:, None]
    return (q >= k).astype(ml_dtypes.bfloat16)


def _in_maps(embedded, Wq, Wk, Wv):
    embedded = np.asarray(embedded, dtype=np.float32)
    Wq = np.asarray(Wq, dtype=np.float32)
    Wk = np.asarray(Wk, dtype=np.float32)
    Wv = np.asarray(Wv, dtype=np.float32)
    mask = _mask_np()
    in_maps = []
    for core in range(8):
        b, h = divmod(core, 2)
        in_maps.append(
            {
                "xt": np.ascontiguousarray(embedded[b].T).astype(ml_dtypes.bfloat16),
                "wq": np.ascontiguousarray(Wq[h]).astype(ml_dtypes.bfloat16),
                "wk": np.ascontiguousarray(Wk[h]).astype(ml_dtypes.bfloat16),
                "wv": np.ascontiguousarray(Wv[h]).astype(ml_dtypes.bfloat16),
                "mask": mask,
            }
        )
    return in_maps


def _gather(results):
    out = np.empty((B, S, H * HD), np.float32)
    for core in range(8):
        b, h = divmod(core, 2)
        out[b, :, h * HD : (h + 1) * HD] = results[core]["out"].astype(np.float32)
    return out


def _get_nc():
    global _NC
    if _NC is None:
        _NC = _build_nc()
    return _NC


def kernel(embedded, Wq, Wk, Wv):
    res = bass_utils.run_bass_kernel_spmd(
        _get_nc(), _in_maps(embedded, Wq, Wk, Wv), core_ids=list(range(8))
    )
    return _gather(res.results)


def kernel_traced(embedded, Wq, Wk, Wv):
    """Like kernel() but with NTFF tracing; returns (out, BassKernelResults)."""
    res = bass_utils.run_bass_kernel_spmd(
        _get_nc(), _in_maps(embedded, Wq, Wk, Wv), core_ids=list(range(8)), trace=True
    )
    return _gather(res.results), res
